# revision 13
# baseline (speedup 1.0000x reference)
"""Trainium2 Bass kernel for the dual-pass (inter/intra) MultiHeadAttention module.

Contract: kernel(**inputs) takes FULL unsharded numpy inputs (keys as in
setup_inputs()) and returns the FULL [32, 512, 512] float32 output.

Sharding: data-parallel over batch. 8 cores x 4 batch elements each; all
weights replicated; no collectives. Host pre-transposes weights, gathers
per-core outputs.

Per-core dataflow (per batch element, all activations kept feature-major
"transposed" [feat, token] so each layer is matmul(lhsT=W^T, rhs=actT)):
  inter:  A1^T = Wp @ x          (x used in natural layout as moving operand)
          A2^T = Wsi @ A1
          per head: qT,kT = Wq/Wk @ A2_h ; v = A2_h^T @ Wv^T (natural [m,e])
          S^T[m,n] = kT^T... = MM(lhsT=kT chunk, rhs=qT);  P^T = exp(S^T/8)
          o_aug^T[65,n] = sum_m MM(lhsT=[v|1][m,65], rhs=P^T[m,n])
          normalize: recip = 1/o_aug[64]; bcast via K=1 matmul; DVE mul
          oi^T = Woi @ concat ; out_inter = (Wpo @ oi)^T + x   (natural!)
  intra:  out_inter^T via PE transpose; xi^T = Wsa @ out_inter^T; same MHA;
          out natural = MM(lhsT=concat2T chunk, rhs=Woa^T*(1-a));
          final = out*(1-a) + a*out_inter  ((1-a) folded into Woa on host)
"""

import os
import sys
from contextlib import ExitStack

import numpy as np

sys.path.insert(0, "/opt/trn_rl_repo")

from concourse import bass, bacc, mybir, tile  # noqa: E402
from concourse.bass_utils import run_bass_kernel_spmd  # noqa: E402

B, S, D = 32, 512, 512
H, HD = 8, 64
NCORES = 8
BPC = B // NCORES  # batches per core
P = 128  # partitions
NT = D // P  # 4 tiles per 512 axis

F32 = mybir.dt.float32
F32R = mybir.dt.float32r

# matmul operand mode: "f32r" (1 cyc/row, relaxed precision) or "f32" (4 cyc/row).
# fp32r operands must be *written* as float32r by a compute engine (the BIR
# verifier rejects DMA-produced fp32r matmul inputs), so DMA'd tensors are
# staged through an on-chip rounding copy and intermediates are written with
# the f32r output dtype directly.
MM_MODE = os.environ.get("BASS_MM_MODE", "f32r")
MDT = F32R if MM_MODE == "f32r" else F32


def _mm(nc, out, lhsT, rhs, start=None, stop=None):
    nc.tensor.matmul(out, lhsT, rhs, start=start, stop=stop)


def build_bass(a_val: float, with_mask: bool):
    """Build the single-core SPMD program. a_val = sigmoid(alpha)."""
    nc = bacc.Bacc(
        "TRN2",
        target_bir_lowering=False,
        debug=False,
        enable_asserts=False,
        num_devices=NCORES,
    )

    x_d = nc.dram_tensor("x", [BPC, S, D], F32, kind="ExternalInput")
    w_names = [
        ("WpT", [S, D]),
        ("WsiT", [D, D]),
        ("WoiT", [D, D]),
        ("WpoT", [D, S]),
        ("WsaT", [D, D]),
        ("WoaT", [D, D]),
        ("WqiT", [H, HD, HD]),
        ("WkiT", [H, HD, HD]),
        ("WviT", [H, HD, HD]),
        ("WqaT", [H, HD, HD]),
        ("WkaT", [H, HD, HD]),
        ("WvaT", [H, HD, HD]),
        ("ident", [P, P]),
    ]
    wd = {n: nc.dram_tensor(n, shp, F32, kind="ExternalInput") for n, shp in w_names}
    if with_mask:
        wd["maskT"] = nc.dram_tensor("maskT", [S, S], F32, kind="ExternalInput")
    y_d = nc.dram_tensor("y", [BPC, S, D], F32, kind="ExternalOutput")

    EXP = mybir.ActivationFunctionType.Exp
    MULT = mybir.AluOpType.mult
    ADD = mybir.AluOpType.add

    with tile.TileContext(nc) as tc, ExitStack() as ctx:
        if MDT != F32:
            # f32r tiles trip the low-precision accumulation guard; all real
            # accumulation here happens in fp32 PSUM.
            ctx.enter_context(
                nc.allow_low_precision(reason="fp32r-rounded matmul operands")
            )
        wpool = ctx.enter_context(tc.tile_pool(name="weights", bufs=1))
        apool = ctx.enter_context(tc.tile_pool(name="acts", bufs=1))
        dpool = ctx.enter_context(tc.tile_pool(name="dbuf", bufs=2))
        pspool = ctx.enter_context(tc.tile_pool(name="psum", bufs=7, space="PSUM"))

        def ps(shape, tag="ps"):
            return pspool.tile(shape, F32, tag=tag, name=tag)

        # ---- persistent weights in SBUF ----
        def load_big(name, dram, dt=None):  # [512,512] -> 4 x [128,512]
            dt = MDT if dt is None else dt
            tiles = []
            for k in range(NT):
                t = wpool.tile([P, 512], dt, tag=f"{name}{k}", name=f"{name}{k}")
                if dt == F32:
                    nc.sync.dma_start(t[:], dram[k * P : (k + 1) * P, :])
                else:
                    stg = dpool.tile([P, 512], F32, tag="wstage", name="wstage")
                    nc.sync.dma_start(stg[:], dram[k * P : (k + 1) * P, :])
                    nc.vector.tensor_copy(t[:], stg[:])
                tiles.append(t)
            return tiles

        wpT = load_big("WpT", wd["WpT"])
        wsiT = load_big("WsiT", wd["WsiT"])
        woiT = load_big("WoiT", wd["WoiT"])
        wpoT = load_big("WpoT", wd["WpoT"])
        wsaT = load_big("WsaT", wd["WsaT"])
        woaT = load_big("WoaT", wd["WoaT"])

        def load_heads(name, dram):
            """[8,64,64] -> 4 x [128,64] tiles (two heads stacked per tile).
            Returns per-head APs sliced so head h sits at base partition
            (h%2)*64, matching the activation slice it pairs with."""
            tiles = []
            for g in range(H // 2):
                t = wpool.tile([P, HD], MDT, tag=f"{name}{g}", name=f"{name}{g}")
                if MDT == F32:
                    nc.sync.dma_start(
                        t[:], dram[2 * g : 2 * g + 2].rearrange("a b c -> (a b) c")
                    )
                else:
                    stg = dpool.tile([P, HD], F32, tag="hstage", name="hstage")
                    nc.sync.dma_start(
                        stg[:], dram[2 * g : 2 * g + 2].rearrange("a b c -> (a b) c")
                    )
                    nc.vector.tensor_copy(t[:], stg[:])
                tiles.append(t)
            return [
                tiles[h // 2][(h % 2) * HD : (h % 2) * HD + HD, :] for h in range(H)
            ]

        wqi = load_heads("WqiT", wd["WqiT"])
        wki = load_heads("WkiT", wd["WkiT"])
        wvi = load_heads("WviT", wd["WviT"])
        wqa = load_heads("WqaT", wd["WqaT"])
        wka = load_heads("WkaT", wd["WkaT"])
        wva = load_heads("WvaT", wd["WvaT"])

        ident = wpool.tile([P, P], F32, tag="ident", name="ident")
        nc.sync.dma_start(ident[:], wd["ident"][:])
        ones_f32 = wpool.tile([P, 1], F32, tag="ones_f32", name="ones_f32")
        nc.vector.memset(ones_f32[:], 1.0)
        ones64 = wpool.tile([1, HD], MDT, tag="ones64", name="ones64")
        if MDT == F32:
            nc.vector.memset(ones64[:], 1.0)
        else:
            # rounding producer for f32r (memset can't write f32r)
            nc.vector.tensor_copy(
                ones64[:], ones_f32[0:1, 0:1].broadcast_to([1, HD])
            )

        maskT = None
        if with_mask:
            maskT = load_big("maskT", wd["maskT"])

        # ---- helpers ----
        def chain512(lhsT_tiles, rhs_tiles, out_tag):
            """out^T[m-chunk] = sum_k lhsT_tiles[k][:, m]^T @ rhs_tiles[k].
            Returns 4 x [128, 512] SBUF tiles."""
            outs = []
            for m in range(NT):
                acc = ps([P, 512])
                for k in range(NT):
                    _mm(
                        nc,
                        acc[:],
                        lhsT_tiles[k][:, m * P : (m + 1) * P],
                        rhs_tiles[k][:],
                        start=(k == 0),
                        stop=(k == NT - 1),
                    )
                o = apool.tile([P, 512], MDT, tag=f"{out_tag}{m}", name=f"{out_tag}{m}")
                nc.vector.tensor_copy(o[:], acc[:])
                outs.append(o)
            return outs

        def mha(inT, wq, wk, wv, concat_tag, use_mask):
            """inT: 4 x [128,512] transposed activations [(h,e), n].
            Returns concatT: 4 x [128,512] [(h,e), n]."""
            concatT = [
                apool.tile(
                    [P, 512], MDT, tag=f"{concat_tag}{g}", name=f"{concat_tag}{g}"
                )
                for g in range(NT)
            ]
            for h in range(H):
                src = inT[h // 2][(h % 2) * HD : (h % 2) * HD + HD, :]  # [64,512]
                # qT, kT
                pq = ps([HD, 512])
                _mm(nc, pq[:], wq[h][:], src)
                qT = dpool.tile([HD, 512], MDT, tag="qT", name="qT")
                nc.vector.tensor_copy(qT[:], pq[:])
                pk = ps([HD, 512])
                _mm(nc, pk[:], wk[h][:], src)
                kT = dpool.tile([HD, 512], MDT, tag="kT", name="kT")
                nc.vector.tensor_copy(kT[:], pk[:])
                # v (natural [m, e]) augmented with ones column
                vt = []
                for mc in range(NT):
                    pv = ps([P, HD])
                    _mm(nc, pv[:], src[:, mc * P : (mc + 1) * P], wv[h][:])
                    v = dpool.tile([P, HD + 1], MDT, tag=f"v{mc}", name=f"v{mc}")
                    nc.vector.tensor_copy(v[:, 0:HD], pv[:])
                    nc.vector.tensor_copy(v[:, HD : HD + 1], ones_f32[:])
                    vt.append(v)
                # S^T chunks -> P^T = exp(S^T / 8)
                pts = []
                for mc in range(NT):
                    s_ps = ps([P, 512])
                    _mm(nc, s_ps[:], kT[:, mc * P : (mc + 1) * P], qT[:])
                    pt = dpool.tile([P, 512], MDT, tag=f"pt{mc}", name=f"pt{mc}")
                    if use_mask:
                        tmp = dpool.tile([P, 512], F32, tag=f"mtmp{mc}", name=f"mtmp{mc}")
                        nc.vector.scalar_tensor_tensor(
                            tmp[:], s_ps[:], 0.125, maskT[mc][:], MULT, ADD
                        )
                        nc.scalar.activation(pt[:], tmp[:], EXP)
                    else:
                        nc.scalar.activation(pt[:], s_ps[:], EXP, scale=0.125)
                    pts.append(pt)
                # o_aug^T [65, n] = sum_m [v|1]^T... accumulate over m-chunks
                po = ps([HD + 1, 512])
                for mc in range(NT):
                    _mm(
                        nc,
                        po[:],
                        vt[mc][:],
                        pts[mc][:],
                        start=(mc == 0),
                        stop=(mc == NT - 1),
                    )
                # normalize: rows 0..63 divided by row 64 (broadcast over free dim)
                rec = dpool.tile([1, 512], MDT, tag="rec", name="rec")
                nc.vector.reciprocal(rec[:], po[HD : HD + 1, :])
                pb = ps([HD, 512])
                _mm(nc, pb[:], ones64[:], rec[:])
                bc = dpool.tile([HD, 512], F32, tag="bc", name="bc")
                nc.scalar.copy(bc[:], pb[:])
                nc.vector.tensor_mul(
                    concatT[h // 2][(h % 2) * HD : (h % 2) * HD + HD, :],
                    po[0:HD, :],
                    bc[:],
                )
            return concatT

        # ---- per-batch pipeline ----
        for b in range(BPC):
            x_sb = []
            x_r = []
            for m in range(NT):
                t = apool.tile([P, 512], F32, tag=f"x{m}", name=f"x{m}")
                nc.sync.dma_start(t[:], x_d[b, m * P : (m + 1) * P, :])
                x_sb.append(t)
                if MDT == F32:
                    x_r.append(t)
                else:
                    tr = apool.tile([P, 512], MDT, tag=f"xr{m}", name=f"xr{m}")
                    nc.vector.tensor_copy(tr[:], t[:])
                    x_r.append(tr)

            a1T = chain512(wpT, x_r, "a1T")  # [d', d]
            a2T = chain512(wsiT, a1T, "a2T")  # [e, d]
            cT = mha(a2T, wqi, wki, wvi, "cT", use_mask=False)
            oiT = chain512(woiT, cT, "oiT")  # [f, d]

            # out_inter natural [s, d] = (Wpo @ oi)^T + x
            oi_n = []
            for m in range(NT):
                acc = ps([P, 512])
                for k in range(NT):
                    _mm(
                        nc,
                        acc[:],
                        wpoT[k][:, m * P : (m + 1) * P],
                        oiT[k][:],
                        start=(k == 0),
                        stop=(k == NT - 1),
                    )
                o = apool.tile([P, 512], F32, tag=f"oint{m}", name=f"oint{m}")
                nc.vector.tensor_add(o[:], acc[:], x_sb[m][:])
                oi_n.append(o)

            # transpose out_inter -> [d, s]
            oiT_t = [
                apool.tile([P, 512], MDT, tag=f"ointT{i}", name=f"ointT{i}")
                for i in range(NT)
            ]
            for i in range(NT):
                for j in range(NT):
                    pt_ps = ps([P, P])
                    nc.tensor.transpose(
                        pt_ps[:], oi_n[j][:, i * P : (i + 1) * P], ident[:]
                    )
                    nc.vector.tensor_copy(oiT_t[i][:, j * P : (j + 1) * P], pt_ps[:])

            xiT = chain512(wsaT, oiT_t, "xiT")  # [e, s]
            c2T = mha(xiT, wqa, wka, wva, "c2T", use_mask=with_mask)

            # out natural [s, f] accumulate over c; Woa pre-scaled by (1-a)
            for m in range(NT):
                acc = ps([P, 512])
                for k in range(NT):
                    _mm(
                        nc,
                        acc[:],
                        c2T[k][:, m * P : (m + 1) * P],
                        woaT[k][:],
                        start=(k == 0),
                        stop=(k == NT - 1),
                    )
                fin = apool.tile([P, 512], F32, tag=f"fin{m}", name=f"fin{m}")
                # fin = out*(1-a) [already folded] + a*out_inter
                nc.vector.scalar_tensor_tensor(
                    fin[:], oi_n[m][:], float(a_val), acc[:], MULT, ADD
                )
                nc.sync.dma_start(y_d[b, m * P : (m + 1) * P, :], fin[:])

    nc.compile()
    return nc


def _prep_inputs(inputs):
    """Host-side: sigmoid(alpha), weight transposes, per-core input maps."""
    f32 = np.float32

    def t2(w):  # [out,in] -> [in,out]
        return np.ascontiguousarray(np.asarray(w, f32).T)

    def t3(w):  # [h,out,in] -> [h,in,out]
        return np.ascontiguousarray(np.transpose(np.asarray(w, f32), (0, 2, 1)))

    a_val = float(1.0 / (1.0 + np.exp(-np.float32(inputs["alpha"]))))
    mask = np.asarray(inputs["mask"], f32)
    with_mask = bool(np.any(mask))

    common = {
        "WpT": t2(inputs["W_proj_in"]),
        "WsiT": t2(inputs["W_split_inter"]),
        "WoiT": t2(inputs["W_out_inter"]),
        "WpoT": t2(inputs["W_proj_out"]),
        "WsaT": t2(inputs["W_split_intra"]),
        "WoaT": np.ascontiguousarray(
            (np.asarray(inputs["W_out_intra"], f32) * f32(1.0 - a_val)).T
        ),
        "WqiT": t3(inputs["Wq_inter"]),
        "WkiT": t3(inputs["Wk_inter"]),
        "WviT": t3(inputs["Wv_inter"]),
        "WqaT": t3(inputs["Wq_intra"]),
        "WkaT": t3(inputs["Wk_intra"]),
        "WvaT": t3(inputs["Wv_intra"]),
        "ident": np.eye(P, dtype=f32),
    }
    if with_mask:
        common["maskT"] = np.ascontiguousarray(mask.T)

    x = np.asarray(inputs["x"], f32)
    in_maps = []
    for c in range(NCORES):
        m = dict(common)
        m["x"] = np.ascontiguousarray(x[c * BPC : (c + 1) * BPC])
        in_maps.append(m)
    return a_val, with_mask, in_maps


def _run(inputs, trace=False):
    a_val, with_mask, in_maps = _prep_inputs(inputs)
    nc = build_bass(a_val, with_mask)
    res = run_bass_kernel_spmd(
        nc,
        in_maps,
        core_ids=list(range(NCORES)),
        trace=trace,
    )
    out = np.concatenate([res.results[c]["y"] for c in range(NCORES)], axis=0)
    return out.astype(np.float32), res


def kernel(**inputs):
    out, _ = _run(inputs, trace=False)
    return out


# revision 15
# speedup vs baseline: 2.4847x; 2.4847x over previous
"""Trainium2 Bass kernel for the dual-pass (inter/intra) MultiHeadAttention module.

Contract: kernel(**inputs) takes FULL unsharded numpy inputs (keys as in
setup_inputs()) and returns the FULL [32, 512, 512] float32 output.

Sharding: data-parallel over batch. 8 cores x 4 batch elements each; all
weights replicated; no collectives. Host pre-transposes weights, gathers
per-core outputs.

Per-core dataflow (per batch element, all activations kept feature-major
"transposed" [feat, token] so each layer is matmul(lhsT=W^T, rhs=actT)):
  inter:  A1^T = Wp @ x          (x used in natural layout as moving operand)
          A2^T = Wsi @ A1
          per head: qT,kT = Wq/Wk @ A2_h ; v = A2_h^T @ Wv^T (natural [m,e])
          S^T[m,n] = kT^T... = MM(lhsT=kT chunk, rhs=qT);  P^T = exp(S^T/8)
          o_aug^T[65,n] = sum_m MM(lhsT=[v|1][m,65], rhs=P^T[m,n])
          normalize: recip = 1/o_aug[64]; bcast via K=1 matmul; DVE mul
          oi^T = Woi @ concat ; out_inter = (Wpo @ oi)^T + x   (natural!)
  intra:  out_inter^T via PE transpose; xi^T = Wsa @ out_inter^T; same MHA;
          out natural = MM(lhsT=concat2T chunk, rhs=Woa^T*(1-a));
          final = out*(1-a) + a*out_inter  ((1-a) folded into Woa on host)
"""

import os
import sys
from contextlib import ExitStack

import numpy as np

sys.path.insert(0, "/opt/trn_rl_repo")

from concourse import bass, bacc, mybir, tile  # noqa: E402
from concourse.bass_utils import run_bass_kernel_spmd  # noqa: E402

B, S, D = 32, 512, 512
H, HD = 8, 64
NCORES = 8
BPC = B // NCORES  # batches per core
P = 128  # partitions
NT = D // P  # 4 tiles per 512 axis

F32 = mybir.dt.float32
F32R = mybir.dt.float32r

# matmul operand mode: "f32r" (1 cyc/row, relaxed precision) or "f32" (4 cyc/row).
# fp32r operands must be *written* as float32r by a compute engine (the BIR
# verifier rejects DMA-produced fp32r matmul inputs), so DMA'd tensors are
# staged through an on-chip rounding copy and intermediates are written with
# the f32r output dtype directly.
MM_MODE = os.environ.get("BASS_MM_MODE", "f32r")
MDT = F32R if MM_MODE == "f32r" else F32
# test-only knob: repeat the per-batch pipeline N times (for differential timing)
REPEAT = int(os.environ.get("BASS_REPEAT", "1"))


def _mm(nc, out, lhsT, rhs, start=None, stop=None):
    nc.tensor.matmul(out, lhsT, rhs, start=start, stop=stop)


def build_bass(a_val: float, with_mask: bool):
    """Build the single-core SPMD program. a_val = sigmoid(alpha)."""
    nc = bacc.Bacc(
        "TRN2",
        target_bir_lowering=False,
        debug=False,
        enable_asserts=False,
        num_devices=NCORES,
    )

    x_d = nc.dram_tensor("x", [BPC, S, D], F32, kind="ExternalInput")
    w_names = [
        ("WpT", [S, D]),
        ("WsiT", [D, D]),
        ("WoiT", [D, D]),
        ("WpoT", [D, S]),
        ("WsaT", [D, D]),
        ("WoaT", [D, D]),
        ("WqiT", [H, HD, HD]),
        ("WkiT", [H, HD, HD]),
        ("WviT", [H, HD, HD]),
        ("WqaT", [H, HD, HD]),
        ("WkaT", [H, HD, HD]),
        ("WvaT", [H, HD, HD]),
        ("ident", [P, P]),
    ]
    wd = {n: nc.dram_tensor(n, shp, F32, kind="ExternalInput") for n, shp in w_names}
    if with_mask:
        wd["maskT"] = nc.dram_tensor("maskT", [S, S], F32, kind="ExternalInput")
    y_d = nc.dram_tensor("y", [BPC, S, D], F32, kind="ExternalOutput")

    EXP = mybir.ActivationFunctionType.Exp
    MULT = mybir.AluOpType.mult
    ADD = mybir.AluOpType.add

    with tile.TileContext(nc) as tc, ExitStack() as ctx:
        if MDT != F32:
            # f32r tiles trip the low-precision accumulation guard; all real
            # accumulation here happens in fp32 PSUM.
            ctx.enter_context(
                nc.allow_low_precision(reason="fp32r-rounded matmul operands")
            )
        wpool = ctx.enter_context(tc.tile_pool(name="weights", bufs=1))
        apool = ctx.enter_context(tc.tile_pool(name="acts", bufs=1))
        dpool = ctx.enter_context(tc.tile_pool(name="dbuf", bufs=2))
        pspool = ctx.enter_context(tc.tile_pool(name="psum", bufs=7, space="PSUM"))

        def ps(shape, tag="ps"):
            return pspool.tile(shape, F32, tag=tag, name=tag)

        # ---- persistent weights in SBUF ----
        def load_big(name, dram, dt=None):  # [512,512] -> 4 x [128,512]
            dt = MDT if dt is None else dt
            tiles = []
            for k in range(NT):
                t = wpool.tile([P, 512], dt, tag=f"{name}{k}", name=f"{name}{k}")
                if dt == F32:
                    nc.sync.dma_start(t[:], dram[k * P : (k + 1) * P, :])
                else:
                    stg = dpool.tile([P, 512], F32, tag="wstage", name="wstage")
                    nc.sync.dma_start(stg[:], dram[k * P : (k + 1) * P, :])
                    nc.vector.tensor_copy(t[:], stg[:])
                tiles.append(t)
            return tiles

        wpT = load_big("WpT", wd["WpT"])
        wsiT = load_big("WsiT", wd["WsiT"])
        woiT = load_big("WoiT", wd["WoiT"])
        wpoT = load_big("WpoT", wd["WpoT"])
        wsaT = load_big("WsaT", wd["WsaT"])
        woaT = load_big("WoaT", wd["WoaT"])

        def load_heads(name, dram):
            """[8,64,64] -> 4 x [128,64] tiles (two heads stacked per tile).
            Returns per-head APs sliced so head h sits at base partition
            (h%2)*64, matching the activation slice it pairs with."""
            tiles = []
            for g in range(H // 2):
                t = wpool.tile([P, HD], MDT, tag=f"{name}{g}", name=f"{name}{g}")
                if MDT == F32:
                    nc.sync.dma_start(
                        t[:], dram[2 * g : 2 * g + 2].rearrange("a b c -> (a b) c")
                    )
                else:
                    stg = dpool.tile([P, HD], F32, tag="hstage", name="hstage")
                    nc.sync.dma_start(
                        stg[:], dram[2 * g : 2 * g + 2].rearrange("a b c -> (a b) c")
                    )
                    nc.vector.tensor_copy(t[:], stg[:])
                tiles.append(t)
            return [
                tiles[h // 2][(h % 2) * HD : (h % 2) * HD + HD, :] for h in range(H)
            ]

        wqi = load_heads("WqiT", wd["WqiT"])
        wki = load_heads("WkiT", wd["WkiT"])
        wvi = load_heads("WviT", wd["WviT"])
        wqa = load_heads("WqaT", wd["WqaT"])
        wka = load_heads("WkaT", wd["WkaT"])
        wva = load_heads("WvaT", wd["WvaT"])

        ident = wpool.tile([P, P], F32, tag="ident", name="ident")
        nc.sync.dma_start(ident[:], wd["ident"][:])
        ones_f32 = wpool.tile([P, 1], F32, tag="ones_f32", name="ones_f32")
        nc.vector.memset(ones_f32[:], 1.0)
        ones64 = wpool.tile([1, HD], MDT, tag="ones64", name="ones64")
        if MDT == F32:
            nc.vector.memset(ones64[:], 1.0)
        else:
            # rounding producer for f32r (memset can't write f32r)
            nc.vector.tensor_copy(
                ones64[:], ones_f32[0:1, 0:1].broadcast_to([1, HD])
            )

        maskT = None
        if with_mask:
            maskT = load_big("maskT", wd["maskT"])

        # ---- helpers ----
        def chain512(lhsT_tiles, rhs_tiles, out_tag):
            """out^T[m-chunk] = sum_k lhsT_tiles[k][:, m]^T @ rhs_tiles[k].
            Returns 4 x [128, 512] SBUF tiles."""
            outs = []
            for m in range(NT):
                acc = ps([P, 512])
                for k in range(NT):
                    _mm(
                        nc,
                        acc[:],
                        lhsT_tiles[k][:, m * P : (m + 1) * P],
                        rhs_tiles[k][:],
                        start=(k == 0),
                        stop=(k == NT - 1),
                    )
                o = apool.tile([P, 512], MDT, tag=f"{out_tag}{m}", name=f"{out_tag}{m}")
                nc.vector.tensor_copy(o[:], acc[:])
                outs.append(o)
            return outs

        def mha(inT, wq, wk, wv, concat_tag, use_mask):
            """inT: 4 x [128,512] transposed activations [(h,e), n].
            Returns concatT: 4 x [128,512] [(h,e), n]."""
            concatT = [
                apool.tile(
                    [P, 512], MDT, tag=f"{concat_tag}{g}", name=f"{concat_tag}{g}"
                )
                for g in range(NT)
            ]
            for h in range(H):
                src = inT[h // 2][(h % 2) * HD : (h % 2) * HD + HD, :]  # [64,512]
                # qT, kT
                pq = ps([HD, 512])
                _mm(nc, pq[:], wq[h][:], src)
                qT = dpool.tile([HD, 512], MDT, tag="qT", name="qT")
                nc.vector.tensor_copy(qT[:], pq[:])
                pk = ps([HD, 512])
                _mm(nc, pk[:], wk[h][:], src)
                kT = dpool.tile([HD, 512], MDT, tag="kT", name="kT")
                nc.vector.tensor_copy(kT[:], pk[:])
                # v (natural [m, e]) augmented with ones column
                vt = []
                for mc in range(NT):
                    pv = ps([P, HD])
                    _mm(nc, pv[:], src[:, mc * P : (mc + 1) * P], wv[h][:])
                    v = dpool.tile([P, HD + 1], MDT, tag=f"v{mc}", name=f"v{mc}")
                    nc.vector.tensor_copy(v[:, 0:HD], pv[:])
                    nc.vector.tensor_copy(v[:, HD : HD + 1], ones_f32[:])
                    vt.append(v)
                # S^T chunks -> P^T = exp(S^T / 8)
                pts = []
                for mc in range(NT):
                    s_ps = ps([P, 512])
                    _mm(nc, s_ps[:], kT[:, mc * P : (mc + 1) * P], qT[:])
                    pt = dpool.tile([P, 512], MDT, tag=f"pt{mc}", name=f"pt{mc}")
                    if use_mask:
                        tmp = dpool.tile([P, 512], F32, tag=f"mtmp{mc}", name=f"mtmp{mc}")
                        nc.vector.scalar_tensor_tensor(
                            tmp[:], s_ps[:], 0.125, maskT[mc][:], MULT, ADD
                        )
                        nc.scalar.activation(pt[:], tmp[:], EXP)
                    else:
                        nc.scalar.activation(pt[:], s_ps[:], EXP, scale=0.125)
                    pts.append(pt)
                # o_aug^T [65, n] = sum_m [v|1]^T... accumulate over m-chunks
                po = ps([HD + 1, 512])
                for mc in range(NT):
                    _mm(
                        nc,
                        po[:],
                        vt[mc][:],
                        pts[mc][:],
                        start=(mc == 0),
                        stop=(mc == NT - 1),
                    )
                # normalize: rows 0..63 divided by row 64 (broadcast over free dim)
                rec = dpool.tile([1, 512], MDT, tag="rec", name="rec")
                nc.vector.reciprocal(rec[:], po[HD : HD + 1, :])
                pb = ps([HD, 512])
                _mm(nc, pb[:], ones64[:], rec[:])
                bc = dpool.tile([HD, 512], F32, tag="bc", name="bc")
                nc.scalar.copy(bc[:], pb[:])
                nc.vector.tensor_mul(
                    concatT[h // 2][(h % 2) * HD : (h % 2) * HD + HD, :],
                    po[0:HD, :],
                    bc[:],
                )
            return concatT

        # ---- per-batch pipeline ----
        for b in [bb % BPC for bb in range(BPC * REPEAT)]:
            x_sb = []
            x_r = []
            for m in range(NT):
                t = apool.tile([P, 512], F32, tag=f"x{m}", name=f"x{m}")
                nc.sync.dma_start(t[:], x_d[b, m * P : (m + 1) * P, :])
                x_sb.append(t)
                if MDT == F32:
                    x_r.append(t)
                else:
                    tr = apool.tile([P, 512], MDT, tag=f"xr{m}", name=f"xr{m}")
                    nc.vector.tensor_copy(tr[:], t[:])
                    x_r.append(tr)

            a1T = chain512(wpT, x_r, "a1T")  # [d', d]
            a2T = chain512(wsiT, a1T, "a2T")  # [e, d]
            cT = mha(a2T, wqi, wki, wvi, "cT", use_mask=False)
            oiT = chain512(woiT, cT, "oiT")  # [f, d]

            # out_inter natural [s, d] = (Wpo @ oi)^T + x
            oi_n = []
            for m in range(NT):
                acc = ps([P, 512])
                for k in range(NT):
                    _mm(
                        nc,
                        acc[:],
                        wpoT[k][:, m * P : (m + 1) * P],
                        oiT[k][:],
                        start=(k == 0),
                        stop=(k == NT - 1),
                    )
                o = apool.tile([P, 512], F32, tag=f"oint{m}", name=f"oint{m}")
                nc.vector.tensor_add(o[:], acc[:], x_sb[m][:])
                oi_n.append(o)

            # transpose out_inter -> [d, s]
            oiT_t = [
                apool.tile([P, 512], MDT, tag=f"ointT{i}", name=f"ointT{i}")
                for i in range(NT)
            ]
            for i in range(NT):
                for j in range(NT):
                    pt_ps = ps([P, P])
                    nc.tensor.transpose(
                        pt_ps[:], oi_n[j][:, i * P : (i + 1) * P], ident[:]
                    )
                    nc.vector.tensor_copy(oiT_t[i][:, j * P : (j + 1) * P], pt_ps[:])

            xiT = chain512(wsaT, oiT_t, "xiT")  # [e, s]
            c2T = mha(xiT, wqa, wka, wva, "c2T", use_mask=with_mask)

            # out natural [s, f] accumulate over c; Woa pre-scaled by (1-a)
            for m in range(NT):
                acc = ps([P, 512])
                for k in range(NT):
                    _mm(
                        nc,
                        acc[:],
                        c2T[k][:, m * P : (m + 1) * P],
                        woaT[k][:],
                        start=(k == 0),
                        stop=(k == NT - 1),
                    )
                fin = apool.tile([P, 512], F32, tag=f"fin{m}", name=f"fin{m}")
                # fin = out*(1-a) [already folded] + a*out_inter
                nc.vector.scalar_tensor_tensor(
                    fin[:], oi_n[m][:], float(a_val), acc[:], MULT, ADD
                )
                nc.sync.dma_start(y_d[b, m * P : (m + 1) * P, :], fin[:])

    nc.compile()
    return nc


def _prep_inputs(inputs):
    """Host-side: sigmoid(alpha), weight transposes, per-core input maps."""
    f32 = np.float32

    def t2(w):  # [out,in] -> [in,out]
        return np.ascontiguousarray(np.asarray(w, f32).T)

    def t3(w):  # [h,out,in] -> [h,in,out]
        return np.ascontiguousarray(np.transpose(np.asarray(w, f32), (0, 2, 1)))

    a_val = float(1.0 / (1.0 + np.exp(-np.float32(inputs["alpha"]))))
    mask = np.asarray(inputs["mask"], f32)
    with_mask = bool(np.any(mask))

    common = {
        "WpT": t2(inputs["W_proj_in"]),
        "WsiT": t2(inputs["W_split_inter"]),
        "WoiT": t2(inputs["W_out_inter"]),
        "WpoT": t2(inputs["W_proj_out"]),
        "WsaT": t2(inputs["W_split_intra"]),
        "WoaT": np.ascontiguousarray(
            (np.asarray(inputs["W_out_intra"], f32) * f32(1.0 - a_val)).T
        ),
        "WqiT": t3(inputs["Wq_inter"]),
        "WkiT": t3(inputs["Wk_inter"]),
        "WviT": t3(inputs["Wv_inter"]),
        "WqaT": t3(inputs["Wq_intra"]),
        "WkaT": t3(inputs["Wk_intra"]),
        "WvaT": t3(inputs["Wv_intra"]),
        "ident": np.eye(P, dtype=f32),
    }
    if with_mask:
        common["maskT"] = np.ascontiguousarray(mask.T)

    x = np.asarray(inputs["x"], f32)
    in_maps = []
    for c in range(NCORES):
        m = dict(common)
        m["x"] = np.ascontiguousarray(x[c * BPC : (c + 1) * BPC])
        in_maps.append(m)
    return a_val, with_mask, in_maps


def _run(inputs, trace=False):
    a_val, with_mask, in_maps = _prep_inputs(inputs)
    nc = build_bass(a_val, with_mask)
    res = run_bass_kernel_spmd(
        nc,
        in_maps,
        core_ids=list(range(NCORES)),
        trace=trace,
    )
    out = np.concatenate([res.results[c]["y"] for c in range(NCORES)], axis=0)
    return out.astype(np.float32), res


def kernel(**inputs):
    out, _ = _run(inputs, trace=False)
    return out


# revision 19
# speedup vs baseline: 15.9175x; 6.4061x over previous
"""Trainium2 Bass kernel for the dual-pass (inter/intra) MultiHeadAttention module.

Contract: kernel(**inputs) takes FULL unsharded numpy inputs (keys as in
setup_inputs()) and returns the FULL [32, 512, 512] float32 output.

Sharding: data-parallel over batch. 8 cores x 4 batch elements each; all
weights replicated; no collectives. Host pre-transposes weights, gathers
per-core outputs.

Per-core dataflow (per batch element, all activations kept feature-major
"transposed" [feat, token] so each layer is matmul(lhsT=W^T, rhs=actT)):
  inter:  A1^T = Wp @ x          (x used in natural layout as moving operand)
          A2^T = Wsi @ A1
          per head: qT,kT = Wq/Wk @ A2_h ; v = A2_h^T @ Wv^T (natural [m,e])
          S^T[m,n] = kT^T... = MM(lhsT=kT chunk, rhs=qT);  P^T = exp(S^T/8)
          o_aug^T[65,n] = sum_m MM(lhsT=[v|1][m,65], rhs=P^T[m,n])
          normalize: recip = 1/o_aug[64]; bcast via K=1 matmul; DVE mul
          oi^T = Woi @ concat ; out_inter = (Wpo @ oi)^T + x   (natural!)
  intra:  out_inter^T via PE transpose; xi^T = Wsa @ out_inter^T; same MHA;
          out natural = MM(lhsT=concat2T chunk, rhs=Woa^T*(1-a));
          final = out*(1-a) + a*out_inter  ((1-a) folded into Woa on host)
"""

import os
import sys
from contextlib import ExitStack

import numpy as np

sys.path.insert(0, "/opt/trn_rl_repo")

from concourse import bass, bacc, mybir, tile  # noqa: E402
from concourse.bass_utils import run_bass_kernel_spmd  # noqa: E402

B, S, D = 32, 512, 512
H, HD = 8, 64
NCORES = 8
BPC = B // NCORES  # batches per core
P = 128  # partitions
NT = D // P  # 4 tiles per 512 axis

F32 = mybir.dt.float32
F32R = mybir.dt.float32r

# matmul operand mode: "f32r" (1 cyc/row, relaxed precision) or "f32" (4 cyc/row).
# fp32r operands must be *written* as float32r by a compute engine (the BIR
# verifier rejects DMA-produced fp32r matmul inputs), so DMA'd tensors are
# staged through an on-chip rounding copy and intermediates are written with
# the f32r output dtype directly.
MM_MODE = os.environ.get("BASS_MM_MODE", "f32r")
MDT = F32R if MM_MODE == "f32r" else F32
# test-only knob: repeat the per-batch pipeline N times (for differential timing)
REPEAT = int(os.environ.get("BASS_REPEAT", "1"))


def _mm(nc, out, lhsT, rhs, start=None, stop=None):
    nc.tensor.matmul(out, lhsT, rhs, start=start, stop=stop)


def build_bass(a_val: float, with_mask: bool):
    """Build the single-core SPMD program. a_val = sigmoid(alpha)."""
    nc = bacc.Bacc(
        "TRN2",
        target_bir_lowering=False,
        debug=False,
        enable_asserts=False,
        num_devices=NCORES,
    )

    x_d = nc.dram_tensor("x", [BPC, S, D], F32, kind="ExternalInput")
    w_names = [
        ("WpT", [S, D]),
        ("WsiT", [D, D]),
        ("WoiT", [D, D]),
        ("WpoT", [D, S]),
        ("WsaT", [D, D]),
        ("WoaT", [D, D]),
        ("WqPi", [H // 2, P, P]),
        ("WkPi", [H // 2, P, P]),
        ("WvPi", [H // 2, P, P]),
        ("WqPa", [H // 2, P, P]),
        ("WkPa", [H // 2, P, P]),
        ("WvPa", [H // 2, P, P]),
        ("ident", [P, P]),
    ]
    wd = {n: nc.dram_tensor(n, shp, F32, kind="ExternalInput") for n, shp in w_names}
    if with_mask:
        wd["maskT"] = nc.dram_tensor("maskT", [S, S], F32, kind="ExternalInput")
    y_d = nc.dram_tensor("y", [BPC, S, D], F32, kind="ExternalOutput")

    EXP = mybir.ActivationFunctionType.Exp
    MULT = mybir.AluOpType.mult
    ADD = mybir.AluOpType.add

    with tile.TileContext(nc) as tc, ExitStack() as ctx:
        if MDT != F32:
            # f32r tiles trip the low-precision accumulation guard; all real
            # accumulation here happens in fp32 PSUM.
            ctx.enter_context(
                nc.allow_low_precision(reason="fp32r-rounded matmul operands")
            )
        wpool = ctx.enter_context(tc.tile_pool(name="weights", bufs=1))
        apool = ctx.enter_context(tc.tile_pool(name="acts", bufs=1))
        dpool = ctx.enter_context(tc.tile_pool(name="dbuf", bufs=2))
        pspool = ctx.enter_context(tc.tile_pool(name="psum", bufs=7, space="PSUM"))

        # PSUM: 8 banks total; dedicate banks per pipeline stage so
        # successive heads/chunks overlap instead of round-robining one tag.
        def ps(shape, tag="ps", bufs=1):
            return pspool.tile(shape, F32, tag=tag, name=tag, bufs=bufs)

        # ---- persistent weights in SBUF ----
        def load_big(name, dram, dt=None):  # [512,512] -> 4 x [128,512]
            dt = MDT if dt is None else dt
            tiles = []
            for k in range(NT):
                t = wpool.tile([P, 512], dt, tag=f"{name}{k}", name=f"{name}{k}")
                if dt == F32:
                    nc.sync.dma_start(t[:], dram[k * P : (k + 1) * P, :])
                else:
                    stg = dpool.tile([P, 512], F32, tag="wstage", name="wstage")
                    nc.sync.dma_start(stg[:], dram[k * P : (k + 1) * P, :])
                    nc.vector.tensor_copy(t[:], stg[:])
                tiles.append(t)
            return tiles

        wpT = load_big("WpT", wd["WpT"])
        wsiT = load_big("WsiT", wd["WsiT"])
        woiT = load_big("WoiT", wd["WoiT"])
        wpoT = load_big("WpoT", wd["WpoT"])
        wsaT = load_big("WsaT", wd["WsaT"])
        woaT = load_big("WoaT", wd["WoaT"])

        def load_pairs(name, dram):
            """[4,128,128] block-diagonal pair weights -> 4 tiles [128,128]."""
            tiles = []
            for g in range(H // 2):
                t = wpool.tile([P, P], MDT, tag=f"{name}{g}", name=f"{name}{g}")
                if MDT == F32:
                    nc.sync.dma_start(t[:], dram[g])
                else:
                    stg = dpool.tile([P, P], F32, tag="pstage", name="pstage")
                    nc.sync.dma_start(stg[:], dram[g])
                    nc.vector.tensor_copy(t[:], stg[:])
                tiles.append(t)
            return tiles

        wqPi = load_pairs("WqPi", wd["WqPi"])
        wkPi = load_pairs("WkPi", wd["WkPi"])
        wvPi = load_pairs("WvPi", wd["WvPi"])
        wqPa = load_pairs("WqPa", wd["WqPa"])
        wkPa = load_pairs("WkPa", wd["WkPa"])
        wvPa = load_pairs("WvPa", wd["WvPa"])

        ident = wpool.tile([P, P], F32, tag="ident", name="ident")
        nc.sync.dma_start(ident[:], wd["ident"][:])
        ones_f32 = wpool.tile([P, 1], F32, tag="ones_f32", name="ones_f32")
        nc.vector.memset(ones_f32[:], 1.0)
        ones64 = wpool.tile([1, HD], MDT, tag="ones64", name="ones64")
        if MDT == F32:
            nc.vector.memset(ones64[:], 1.0)
        else:
            # rounding producer for f32r (memset can't write f32r)
            nc.vector.tensor_copy(
                ones64[:], ones_f32[0:1, 0:1].broadcast_to([1, HD])
            )

        maskT = None
        if with_mask:
            maskT = load_big("maskT", wd["maskT"])

        # ---- helpers ----
        def chain512(lhsT_tiles, rhs_tiles, out_tag, copy_engine="vector"):
            """out^T[m-chunk] = sum_k lhsT_tiles[k][:, m]^T @ rhs_tiles[k].
            Returns 4 x [128, 512] SBUF tiles."""
            outs = []
            for m in range(NT):
                acc = ps([P, 512], tag="acc", bufs=2)
                for k in range(NT):
                    _mm(
                        nc,
                        acc[:],
                        lhsT_tiles[k][:, m * P : (m + 1) * P],
                        rhs_tiles[k][:],
                        start=(k == 0),
                        stop=(k == NT - 1),
                    )
                o = apool.tile([P, 512], MDT, tag=f"{out_tag}{m}", name=f"{out_tag}{m}")
                if copy_engine == "vector":
                    nc.vector.tensor_copy(o[:], acc[:])
                else:
                    nc.scalar.copy(o[:], acc[:])
                outs.append(o)
            return outs

        def mha(inT, wqP, wkP, wvP, concat_tag, use_mask):
            """inT: 4 x [128,512] transposed activations [(h,e), n].
            Head-pair packing: pair g = heads (2g, 2g+1) lives in inT[g];
            block-diagonal pair weights compute both heads per matmul.
            Returns concatT: 4 x [128,512] [(h,e), n]."""
            concatT = [
                apool.tile(
                    [P, 512], MDT, tag=f"{concat_tag}{g}", name=f"{concat_tag}{g}"
                )
                for g in range(NT)
            ]
            for g in range(H // 2):
                src = inT[g]  # [128, 512] = both heads of the pair
                # qT/kT for both heads: [qTA; qTB], [kTA; kTB]
                pq = ps([P, 512], tag="qb", bufs=2)
                _mm(nc, pq[:], wqP[g][:], src[:])
                qp = dpool.tile([P, 512], MDT, tag="qp", name="qp")
                nc.vector.tensor_copy(qp[:], pq[:])
                pk = ps([P, 512], tag="qb", bufs=2)
                _mm(nc, pk[:], wkP[g][:], src[:])
                kp = dpool.tile([P, 512], MDT, tag="kp", name="kp")
                nc.vector.tensor_copy(kp[:], pk[:])
                # v for both heads: pv4[:, mc*128+c] c<64 head A, c>=64 head B
                pv4 = ps([P, 512], tag="sv", bufs=3)
                for mc in range(NT):
                    _mm(
                        nc,
                        pv4[:, mc * P : (mc + 1) * P],
                        src[:, mc * P : (mc + 1) * P],
                        wvP[g][:],
                    )
                pv4v = pv4[:].rearrange("p (a c) -> p a c", a=NT)
                v4s = []
                for hh in range(2):
                    v4 = dpool.tile(
                        [P, NT, HD + 1], MDT, tag=f"v4{hh}", name=f"v4{hh}"
                    )
                    nc.vector.tensor_copy(
                        v4[:, :, 0:HD], pv4v[:, :, hh * HD : (hh + 1) * HD]
                    )
                    nc.vector.tensor_copy(
                        v4[:, :, HD : HD + 1],
                        ones_f32[:, 0:1].broadcast_to([P, NT, 1]),
                    )
                    v4s.append(v4)
                for hh in range(2):
                    h = 2 * g + hh
                    qT = qp[hh * HD : (hh + 1) * HD, :]
                    kT = kp[hh * HD : (hh + 1) * HD, :]
                    # S^T chunks -> P^T = exp(S^T / 8)
                    pts = []
                    for mc in range(NT):
                        s_ps = ps([P, 512], tag="sv", bufs=3)
                        _mm(nc, s_ps[:], kT[:, mc * P : (mc + 1) * P], qT[:])
                        pt = dpool.tile([P, 512], MDT, tag=f"pt{mc}", name=f"pt{mc}")
                        if use_mask:
                            tmp = dpool.tile(
                                [P, 512], F32, tag=f"mtmp{mc}", name=f"mtmp{mc}"
                            )
                            nc.vector.scalar_tensor_tensor(
                                tmp[:], s_ps[:], 0.125, maskT[mc][:], MULT, ADD
                            )
                            nc.scalar.activation(pt[:], tmp[:], EXP)
                        else:
                            nc.scalar.activation(pt[:], s_ps[:], EXP, scale=0.125)
                        pts.append(pt)
                    # o_aug^T [65, n], accumulate over m-chunks
                    po = ps([HD + 1, 512], tag="o", bufs=2)
                    for mc in range(NT):
                        _mm(
                            nc,
                            po[:],
                            v4s[hh][:, mc, :],
                            pts[mc][:],
                            start=(mc == 0),
                            stop=(mc == NT - 1),
                        )
                    # normalize rows 0..63 by row 64 (broadcast over free dim)
                    rec = dpool.tile([1, 512], MDT, tag="rec", name="rec")
                    nc.vector.reciprocal(rec[:], po[HD : HD + 1, :])
                    pb = ps([HD, 512], tag="qb", bufs=2)
                    _mm(nc, pb[:], ones64[:], rec[:])
                    bc = dpool.tile([HD, 512], F32, tag="bc", name="bc")
                    nc.scalar.copy(bc[:], pb[:])
                    nc.vector.tensor_mul(
                        concatT[g][hh * HD : (hh + 1) * HD, :],
                        po[0:HD, :],
                        bc[:],
                    )
            return concatT

        # ---- per-batch pipeline ----
        for b in [bb % BPC for bb in range(BPC * REPEAT)]:
            x_sb = []
            x_r = []
            for m in range(NT):
                t = apool.tile([P, 512], F32, tag=f"x{m}", name=f"x{m}")
                nc.sync.dma_start(t[:], x_d[b, m * P : (m + 1) * P, :])
                x_sb.append(t)
                if MDT == F32:
                    x_r.append(t)
                else:
                    tr = apool.tile([P, 512], MDT, tag=f"xr{m}", name=f"xr{m}")
                    nc.vector.tensor_copy(tr[:], t[:])
                    x_r.append(tr)

            a1T = chain512(wpT, x_r, "a1T")  # [d', d]
            a2T = chain512(wsiT, a1T, "a2T")  # [e, d]
            cT = mha(a2T, wqPi, wkPi, wvPi, "cT", use_mask=False)
            oiT = chain512(woiT, cT, "oiT", copy_engine="scalar")  # [f, d]

            # out_inter natural [s, d] = (Wpo @ oi)^T + x
            oi_n = []
            for m in range(NT):
                acc = ps([P, 512], tag="acc", bufs=2)
                for k in range(NT):
                    _mm(
                        nc,
                        acc[:],
                        wpoT[k][:, m * P : (m + 1) * P],
                        oiT[k][:],
                        start=(k == 0),
                        stop=(k == NT - 1),
                    )
                o = apool.tile([P, 512], F32, tag=f"oint{m}", name=f"oint{m}")
                nc.vector.tensor_add(o[:], acc[:], x_sb[m][:])
                oi_n.append(o)

            # transpose out_inter -> [d, s]
            oiT_t = [
                apool.tile([P, 512], MDT, tag=f"ointT{i}", name=f"ointT{i}")
                for i in range(NT)
            ]
            for i in range(NT):
                for j in range(NT):
                    pt_ps = ps([P, P], tag="s", bufs=2)
                    nc.tensor.transpose(
                        pt_ps[:], oi_n[j][:, i * P : (i + 1) * P], ident[:]
                    )
                    nc.scalar.copy(oiT_t[i][:, j * P : (j + 1) * P], pt_ps[:])

            xiT = chain512(wsaT, oiT_t, "xiT", copy_engine="scalar")  # [e, s]
            c2T = mha(xiT, wqPa, wkPa, wvPa, "c2T", use_mask=with_mask)

            # out natural [s, f] accumulate over c; Woa pre-scaled by (1-a)
            for m in range(NT):
                acc = ps([P, 512], tag="acc", bufs=2)
                for k in range(NT):
                    _mm(
                        nc,
                        acc[:],
                        c2T[k][:, m * P : (m + 1) * P],
                        woaT[k][:],
                        start=(k == 0),
                        stop=(k == NT - 1),
                    )
                fin = apool.tile([P, 512], F32, tag=f"fin{m}", name=f"fin{m}")
                # fin = out*(1-a) [already folded] + a*out_inter
                nc.vector.scalar_tensor_tensor(
                    fin[:], oi_n[m][:], float(a_val), acc[:], MULT, ADD
                )
                nc.sync.dma_start(y_d[b, m * P : (m + 1) * P, :], fin[:])

    nc.compile()
    return nc


def _prep_inputs(inputs):
    """Host-side: sigmoid(alpha), weight transposes, per-core input maps."""
    f32 = np.float32

    def t2(w):  # [out,in] -> [in,out]
        return np.ascontiguousarray(np.asarray(w, f32).T)

    def t3(w):  # [h,out,in] -> [h,in,out]
        return np.ascontiguousarray(np.transpose(np.asarray(w, f32), (0, 2, 1)))

    def pairblk(w):
        """[8,64,64] per-head W -> [4,128,128] block-diag pair lhsT:
        blkdiag(W[2g].T, W[2g+1].T)."""
        wt = t3(w)
        out = np.zeros((H // 2, P, P), f32)
        for g in range(H // 2):
            out[g, :HD, :HD] = wt[2 * g]
            out[g, HD:, HD:] = wt[2 * g + 1]
        return out

    a_val = float(1.0 / (1.0 + np.exp(-np.float32(inputs["alpha"]))))
    mask = np.asarray(inputs["mask"], f32)
    with_mask = bool(np.any(mask))

    common = {
        "WpT": t2(inputs["W_proj_in"]),
        "WsiT": t2(inputs["W_split_inter"]),
        "WoiT": t2(inputs["W_out_inter"]),
        "WpoT": t2(inputs["W_proj_out"]),
        "WsaT": t2(inputs["W_split_intra"]),
        "WoaT": np.ascontiguousarray(
            (np.asarray(inputs["W_out_intra"], f32) * f32(1.0 - a_val)).T
        ),
        "WqPi": pairblk(inputs["Wq_inter"]),
        "WkPi": pairblk(inputs["Wk_inter"]),
        "WvPi": pairblk(inputs["Wv_inter"]),
        "WqPa": pairblk(inputs["Wq_intra"]),
        "WkPa": pairblk(inputs["Wk_intra"]),
        "WvPa": pairblk(inputs["Wv_intra"]),
        "ident": np.eye(P, dtype=f32),
    }
    if with_mask:
        common["maskT"] = np.ascontiguousarray(mask.T)

    x = np.asarray(inputs["x"], f32)
    in_maps = []
    for c in range(NCORES):
        m = dict(common)
        m["x"] = np.ascontiguousarray(x[c * BPC : (c + 1) * BPC])
        in_maps.append(m)
    return a_val, with_mask, in_maps


def _run(inputs, trace=False):
    a_val, with_mask, in_maps = _prep_inputs(inputs)
    nc = build_bass(a_val, with_mask)
    res = run_bass_kernel_spmd(
        nc,
        in_maps,
        core_ids=list(range(NCORES)),
        trace=trace,
    )
    out = np.concatenate([res.results[c]["y"] for c in range(NCORES)], axis=0)
    return out.astype(np.float32), res


def kernel(**inputs):
    out, _ = _run(inputs, trace=False)
    return out
